# revision 21
# baseline (speedup 1.0000x reference)
"""Trainium2 Bass kernel for nn_AdditiveModel (grouped per-edge MLP + masked lag conv).

Reference computation (B=32768, N=16, L=16, H=16, G=N*N=256):
    xm  = x * (causal != 0)                     # [B, G, L]
    h1  = sigmoid(einsum('bgl,ghl->bgh', xm, W1) + b1)
    h2  = sigmoid(einsum('bgh,gkh->bgk', h1, W2) + b2)
    out = einsum('bvm,vm->bv', h2.reshape(B,N,N*H), W3) + b3   # [B, 16]

Strategy (pure data parallel over 8 NeuronCores, batch-sharded):
  - causal mask folded into W1 on the host (W1' = W1 * mask) -- no on-chip mask op.
  - 8 groups are packed block-diagonally into each 128x128 weight tile, so the
    per-group (16->16) convs become full-width TensorE matmuls over
    K=128 "channels" (= 8 groups x 16 lags/hidden).  32 channel-chunks cover
    all G*L (resp. G*H) = 4096 channels.
  - W3 is folded into a block-structured [128, 16] stationary tile per chunk
    (only column v = chunk//2 is nonzero), so stage 3 is a PSUM-accumulated
    matmul -- no separate elementwise multiply or partition reduction.
  - b1/b2 ride free on ScalarE's activation (out = sigmoid(in + bias)).
  - x is host-pre-transposed per shard to channel-major [4096, 4096] and
    blocked so every x DMA is a fully contiguous 2 MiB transfer.
  - stage-1 matmul runs in float32r (full-rate fp32 PE path); h1/h2/W2/W3 are
    bf16.  PSUM accumulation is fp32 throughout.
  - the trace is software-pipelined (stage1 two iters ahead, stage3 one iter
    behind) so ScalarE -- the bottleneck engine at ~2 sigmoids x 16.8M
    elements per core -- never stalls on TensorE.  Steady state is gapless on
    ScalarE; the only remaining idle is the HBM-bound startup ramp, minimized
    by loading w1/w2 in lazy pieces and splitting the first x tile.
"""

import sys
import time

import numpy as np

import ml_dtypes

if "/opt/trn_rl_repo" not in sys.path:
    sys.path.insert(0, "/opt/trn_rl_repo")

N = 16
L = 16
H = 16
B = 32768
G = N * N                 # 256 groups
NCORES = 8
BS = B // NCORES          # 4096 batch rows per core
C = G * L                 # 4096 channels (also G*H)
NCHUNK = 32               # channel chunks of 128
GRP = 8                   # groups per chunk
NBT = 4                   # batch tiles per core
BT = 1024                 # batch-tile width (columns)
NITER = NBT * NCHUNK      # 128 iterations per core

_graph_cache = {}


def _build_graph():
    """Build + compile the per-core Bass graph (shared SPMD across 8 cores)."""
    from concourse import bacc, tile, mybir

    f32 = mybir.dt.float32
    f32r = mybir.dt.float32r
    bf16 = mybir.dt.bfloat16
    SIG = mybir.ActivationFunctionType.Sigmoid

    nc = bacc.Bacc("TRN2", target_bir_lowering=False, debug=False,
                   num_devices=NCORES)

    # x: [bt, cg, p, (j, col)] -- host-bricked so each [128, 4096] tile is a
    # single fully-contiguous 2 MiB DMA.  8 column-groups (cg) of 4 chunks (j).
    x_ext = nc.declare_dram_parameter("x", [NBT, 8, 128, 4096], f32r, isOutput=False)
    w1_ext = nc.declare_dram_parameter("w1", [128, NCHUNK * 128], f32r, isOutput=False)
    w2_ext = nc.declare_dram_parameter("w2", [128, NCHUNK * 128], bf16, isOutput=False)
    w3_ext = nc.declare_dram_parameter("w3", [128, NCHUNK * 16], bf16, isOutput=False)
    b1_ext = nc.declare_dram_parameter("b1", [128, NCHUNK], f32, isOutput=False)
    b2_ext = nc.declare_dram_parameter("b2", [128, NCHUNK], f32, isOutput=False)
    b3_ext = nc.declare_dram_parameter("b3", [16, 1], f32, isOutput=False)
    out_ext = nc.declare_dram_parameter("out", [16, BS], f32, isOutput=True)

    with tile.TileContext(nc) as tc:
        with (
            tc.tile_pool(name="consts", bufs=1) as cpool,
            tc.tile_pool(name="xin", bufs=6) as xpool,
            tc.tile_pool(name="h1", bufs=3) as h1pool,
            tc.tile_pool(name="h2", bufs=3) as h2pool,
            tc.tile_pool(name="osb", bufs=2) as opool,
            tc.tile_pool(name="ps12", bufs=3, space="PSUM") as ps12pool,
            tc.tile_pool(name="ps3", bufs=1, space="PSUM") as ps3pool,
        ):
            xt = {}        # group idx -> x tile [128, 4096]

            def load_x_early(g):
                gbt, cg = divmod(g, 8)
                t = xpool.tile([128, 4096], f32r, tag="xin", name=f"x_{g}")
                nc.sync.dma_start(t[:], x_ext[gbt, cg])
                xt[g] = t

            # warm the sigmoid ACT table while the first DMAs stream
            warmsrc = cpool.tile([128, 1], f32)
            nc.vector.memset(warmsrc[:], 0.0)
            warm = cpool.tile([128, 1], f32)
            nc.scalar.activation(warm[:], warmsrc[:], SIG)
            # ramp: only the first w1/w2 pieces and the first x parts gate
            # the pipeline start; later pieces stream behind the early x tiles.
            x0_parts = []
            x0a = xpool.tile([128, 1024], f32r, tag="xin", name="x0a")
            nc.sync.dma_start(x0a[:], x_ext[0, 0, :, 0:1024])
            x0_parts.append(x0a)
            w1p = [cpool.tile([128, 8 * 128], f32r, name=f"w1p{i}")
                   for i in range(4)]
            w2p = [cpool.tile([128, 8 * 128], bf16, name=f"w2p{i}")
                   for i in range(4)]
            nc.sync.dma_start(w1p[0][:], w1_ext[:, 0:1024])
            b1sb = cpool.tile([128, NCHUNK], f32)
            nc.sync.dma_start(b1sb[:], b1_ext[:])
            nc.sync.dma_start(w2p[0][:], w2_ext[:, 0:1024])
            b2sb = cpool.tile([128, NCHUNK], f32)
            nc.sync.dma_start(b2sb[:], b2_ext[:])
            for j in range(1, 4):
                xp = xpool.tile([128, 1024], f32r, tag="xin", name=f"x0_{j}")
                nc.sync.dma_start(xp[:], x_ext[0, 0, :, j * 1024:(j + 1) * 1024])
                x0_parts.append(xp)
            xt[0] = x0_parts
            load_x_early(1)
            w3sb = cpool.tile([128, NCHUNK * 16], bf16)
            nc.sync.dma_start(w3sb[:], w3_ext[:])
            b3sb = cpool.tile([16, 1], f32)
            nc.sync.dma_start(b3sb[:], b3_ext[:])

            def w1_of(c):
                return w1p[c // 8][:, (c % 8) * 128:(c % 8 + 1) * 128]

            def w2_of(c):
                return w2p[c // 8][:, (c % 8) * 128:(c % 8 + 1) * 128]

            def load_late_weights(t):
                if t in (1, 2, 3):
                    nc.sync.dma_start(w1p[t][:],
                                      w1_ext[:, t * 1024:(t + 1) * 1024])
                elif t in (4, 5, 6):
                    i = t - 3
                    nc.sync.dma_start(w2p[i][:],
                                      w2_ext[:, i * 1024:(i + 1) * 1024])
            ps1 = {}
            h1d = {}
            h2d = {}
            ps3 = [None] * NBT

            load_x = load_x_early

            def s1mm(t):
                bt, c = divmod(t, NCHUNK)
                g, j = divmod(t, 4)
                if g not in xt:
                    load_x(g)
                xg = xt[g]
                if isinstance(xg, list):
                    rhs_of = lambda h: xg[j][:, h * 512:(h + 1) * 512]
                else:
                    rhs_of = lambda h: xg[:, j * BT + h * 512:
                                          j * BT + (h + 1) * 512]
                ps = ps12pool.tile([128, BT], f32, tag="ps12")
                for h in range(2):
                    nc.tensor.matmul(
                        ps[:, h * 512:(h + 1) * 512],
                        lhsT=w1_of(c),
                        rhs=rhs_of(h),
                        start=True, stop=True,
                    )
                ps1[t] = ps

            def s1act(t):
                bt, c = divmod(t, NCHUNK)
                h1 = h1pool.tile([128, BT], bf16, tag="h1")
                nc.scalar.activation(h1[:], ps1.pop(t)[:], SIG,
                                     bias=b1sb[:, c:c + 1])
                h1d[t] = h1

            def s2(t):
                bt, c = divmod(t, NCHUNK)
                ps = ps12pool.tile([128, BT], f32, tag="ps12")
                h1 = h1d.pop(t)
                for h in range(2):
                    nc.tensor.matmul(
                        ps[:, h * 512:(h + 1) * 512],
                        lhsT=w2_of(c),
                        rhs=h1[:, h * 512:(h + 1) * 512],
                        start=True, stop=True,
                    )
                h2 = h2pool.tile([128, BT], bf16, tag="h2")
                nc.scalar.activation(h2[:], ps[:], SIG, bias=b2sb[:, c:c + 1])
                h2d[t] = h2

            def s3(t):
                bt, c = divmod(t, NCHUNK)
                if c == 0:
                    ps3[bt] = ps3pool.tile([16, BT], f32, tag="ps3", name=f"ps3_{bt}")
                h2 = h2d.pop(t)
                for h in range(2):
                    nc.tensor.matmul(
                        ps3[bt][:, h * 512:(h + 1) * 512],
                        lhsT=w3sb[:, c * 16:(c + 1) * 16],
                        rhs=h2[:, h * 512:(h + 1) * 512],
                        start=(c == 0), stop=(c == NCHUNK - 1),
                    )
                if c == NCHUNK - 1:
                    osb = opool.tile([16, BT], f32, tag="osb")
                    nc.vector.tensor_scalar_add(osb[:], ps3[bt][:],
                                                b3sb[:, 0:1])
                    nc.sync.dma_start(out_ext[:, bt * BT:(bt + 1) * BT],
                                      osb[:])

            # Software pipeline: stage1 runs 2 iterations ahead of stage2 and
            # stage3 trails one behind, keeping ScalarE (the bottleneck)
            # saturated: its stream alternates s1act(t+1), s2-act(t) with both
            # producers already complete.
            s1mm(0)
            s1mm(1)
            s1act(0)
            for t in range(NITER):
                load_late_weights(t)
                if t + 2 < NITER:
                    s1mm(t + 2)
                if t + 1 < NITER:
                    s1act(t + 1)
                s2(t)
                if t >= 1:
                    s3(t - 1)
            s3(NITER - 1)

    nc.compile()
    return nc


def _get_graph():
    if "nc" not in _graph_cache:
        _graph_cache["nc"] = _build_graph()
    return _graph_cache["nc"]


def _prep_shared(causal, W1, b1, W2, b2, W3, b3):
    """Host-side weight packing (replicated across cores)."""
    bf = ml_dtypes.bfloat16
    mask = (np.asarray(causal).reshape(G, L) != 0).astype(np.float32)
    W1m = np.asarray(W1, dtype=np.float32) * mask[:, None, :]   # [G, H, L]

    def blockdiag(blk):
        # blk: [G, K_in=16, M_out=16] -> [128 (gl*16+k), NCHUNK*128 (c*128+m)]
        bd = np.zeros((NCHUNK, GRP, 16, GRP, 16), dtype=np.float32)
        b5 = blk.reshape(NCHUNK, GRP, 16, 16)
        for gl in range(GRP):
            bd[:, gl, :, gl, :] = b5[:, gl]
        return np.ascontiguousarray(
            bd.reshape(NCHUNK, 128, 128).transpose(1, 0, 2).reshape(128, -1))

    w1h = blockdiag(W1m.transpose(0, 2, 1))                     # k=lag, m=h
    w2h = blockdiag(
        np.asarray(W2, dtype=np.float32).transpose(0, 2, 1)).astype(bf)

    W3f = np.asarray(W3, dtype=np.float32)                      # [N, N*H]
    w3bd = np.zeros((NCHUNK, 128, 16), dtype=np.float32)
    for c in range(NCHUNK):
        w3bd[c, :, c // 2] = W3f[c // 2, (c % 2) * 128:(c % 2) * 128 + 128]
    w3h = np.ascontiguousarray(
        w3bd.transpose(1, 0, 2).reshape(128, NCHUNK * 16)).astype(bf)

    b1h = np.ascontiguousarray(
        np.asarray(b1, dtype=np.float32).reshape(NCHUNK, 128).T)
    b2h = np.ascontiguousarray(
        np.asarray(b2, dtype=np.float32).reshape(NCHUNK, 128).T)
    b3h = np.ascontiguousarray(
        np.asarray(b3, dtype=np.float32).reshape(16, 1))
    return w1h, w2h, w3h, b1h, b2h, b3h


def _prep_x_shard(x_shard):
    """[BS, G, L] -> bricked channel-major [NBT, 8, 128, 4096] (contiguous)."""
    xs = np.asarray(x_shard, dtype=np.float32).reshape(BS, C).T  # [C, BS]
    x5 = xs.reshape(8, 4, 128, NBT, BT)       # [cg, j, p, bt, col]
    return np.ascontiguousarray(x5.transpose(3, 0, 2, 1, 4)
                                .reshape(NBT, 8, 128, 4 * BT))


def _run(inputs, trace=False, trace_cores=None):
    from concourse.bass_utils import run_bass_kernel_spmd

    nc = _get_graph()
    w1h, w2h, w3h, b1h, b2h, b3h = _prep_shared(
        inputs["causal"], inputs["W1"], inputs["b1"], inputs["W2"],
        inputs["b2"], inputs["W3"], inputs["b3"])
    x = np.asarray(inputs["x"], dtype=np.float32)
    in_maps = []
    for i in range(NCORES):
        in_maps.append({
            "x": _prep_x_shard(x[i * BS:(i + 1) * BS]),
            "w1": w1h, "w2": w2h, "w3": w3h,
            "b1": b1h, "b2": b2h, "b3": b3h,
        })
    res = None
    last_err = None
    for attempt in range(3):
        try:
            res = run_bass_kernel_spmd(
                nc, in_maps, list(range(NCORES)),
                trace=trace, trace_cores=trace_cores)
            break
        except Exception as e:  # transient NRT device wedge heals on rerun
            last_err = e
            time.sleep(2.0)
    if res is None:
        raise last_err
    out = np.empty((B, N), dtype=np.float32)
    for i in range(NCORES):
        out[i * BS:(i + 1) * BS] = res.results[i]["out"].T
    return out, res


def kernel(**inputs) -> np.ndarray:
    out, _ = _run(inputs, trace=False)
    return out
